# revision 48
# baseline (speedup 1.0000x reference)
"""Single-head causal attention with tanh soft-capping on 8 TRN2 NeuronCores.

Problem: nn_Attention_30056181138106
  input [8, 2048, 1024] f32, attention_mask [8, 2048] i32 (ones),
  W_Q/W_K/W_V [128, 1024] f32.
  out[b] = softmax(causal_mask(30*tanh((x Wq^T)(x Wk^T)^T / sqrt(128)))) @ (x Wv^T)

Sharding: data-parallel over batch, one batch element per core, weights
replicated. No collectives needed.

Per-core algorithm (transposed-score formulation), q-chunks of 256 columns,
globally software-pipelined at score-group granularity:
  prep(c): x rows DMA'd [128,2,1024]; PE-transpose to xT (f32r data and
           identity: 1.5 cyc/col vs 2.0 for plain f32), DVE evacuates
           PSUM->SBUF; QT/KT = W-proj [dh, L] (f32r matmuls, full PE rate
           at moving dim 256, PSUM-accumulated over 8 dm tiles); V
           projected then PE-transposed back to natural [L, dh].
  SA(c,g): score matmuls for up to ab=4 k-tiles -> PSUM [k=128, 4, 256],
           FULL tiles (tanh caps the causal-wedge garbage, so no stale
           PSUM and batched activations); tanh IN PLACE on PSUM, then exp
           (scale=30, bias=-30) into an SBUF e-tile ring. The deep ring
           (e_bufs) decouples ScalarE from the consumers.
  BW(c,g): deferred until the ring is `lag` groups deep, so ScalarE never
           starves. Diagonal tiles get one GPSIMD 0/1-mask multiply
           (zeroes wedge + masked prefix); AV accumulates [dh, 256]; the
           softmax denominator accumulates TRANSPOSED via mini-matmuls
           (lhsT=E_sub[128k,128q], rhs=ones[128,2] -> [q,2], ~8 cycles
           each; fp32r PSUM writes must be even-sized runs) into the same
           PSUM bank as po: one accumulation group (start on first AV,
           stop on last mini; zero-regions are bank-granular).
  out(c):  reciprocal of den^T on DVE (already q-major), per-q-tile
           PSUM-evacuate + PE-transpose + per-partition scalar multiply,
           DMA out on the GPSIMD/SWDGE queue (SP queue is saturated by x
           loads).
Emission interleaves chunk c's proj/SA pieces with chunk c+1's transposes
so the PE->DVE->PE chains of adjacent chunks overlap; weight DMAs ride the
ACT hwdge queue; the first x load is split per 128 rows to start the PE
early.

Softmax max-subtraction is unnecessary: scores are capped to [-30, 30] by
tanh, so exp(s-30) is in (0, 1] and all sums stay in fp32 range. float32r
matmuls measure ~1.6e-4 max matmul error on TRN2 (better than fp16/bf16);
measured end-to-end rel err 1.03e-2 vs the fp32 reference (gate 2e-2),
dominated by score rounding on short early rows.

The batched-ACT fast path assumes attention_mask is all ones (bias is the
constant -30); kernel() checks the mask at run time and falls back to a
per-k-tile-bias variant when any key is masked (CoreSim-validated).

Cost-model schedule: 66265 ns/core (baseline 115947); PE busy ~53us of
the ~66us span, ScalarE ~44us, DVE ~48us.
"""

import numpy as np
from math import sqrt
from contextlib import ExitStack

import concourse.bass as bass
import concourse.mybir as mybir
from concourse import bacc
from concourse.tile import TileContext
from concourse.bass_utils import run_bass_kernel_spmd
from concourse.masks import make_identity

B, L, DM, DH = 8, 2048, 1024, 128
TAU = 30.0
ISQ = 1.0 / sqrt(DH)
NEG_BIAS = -10000.0  # exp(x + NEG_BIAS) == 0.0 for any capped x

F32 = mybir.dt.float32
F32R = mybir.dt.float32r
BF16 = mybir.dt.bfloat16
I32 = mybir.dt.int32
AF = mybir.ActivationFunctionType

QC = 256            # q-chunk width
NQC = L // QC       # 8 q-chunks
TPC = QC // 128     # 2 q-tiles per chunk
NLT = L // 128      # 16 l-tiles
NDT = DM // 128     # 8 dm-tiles

_CACHE = {}
_BUILD_OPTS = {}


def _build_nc(mask_ones: bool, ab: int = 4, mm_bufs: int = 2, tr_bufs: int = 3,
              acc_bufs: int = 1, xs_bufs: int = 3, e_bufs: int = 11,
              t_bufs: int = 2, outp_bufs: int = 3, pool_wedge: bool = True,
              inplace_exp: bool = True, lag: int = 8, work_bufs: int = 2):
    # Bacc (not bare Bass): its finalize() runs move_matmul_waits_to_ldweights
    # + generate_event_semaphores, required by walrus wait-count limits.
    nc = bacc.Bacc(None, target_bir_lowering=False)
    # f32r relabel of the f32 inputs: same bits, full-rate PE matmuls.
    x = nc.declare_dram_parameter("x", [L, DM], F32R, isOutput=False)
    am = nc.declare_dram_parameter("attention_mask", [L], I32, isOutput=False)
    wq = nc.declare_dram_parameter("W_Q", [DH, DM], F32R, isOutput=False)
    wk = nc.declare_dram_parameter("W_K", [DH, DM], F32R, isOutput=False)
    wv = nc.declare_dram_parameter("W_V", [DH, DM], F32R, isOutput=False)
    out = nc.declare_dram_parameter("out", [L, DH], F32, isOutput=True)

    def ngroups(c):
        return -(-(TPC * c + TPC) // ab)

    with TileContext(nc) as tc:
        with ExitStack() as ctx:
            sb = ctx.enter_context(tc.tile_pool(name="sb", bufs=1))
            stage = ctx.enter_context(tc.tile_pool(name="stage", bufs=xs_bufs))
            tpool = ctx.enter_context(tc.tile_pool(name="tpool", bufs=t_bufs))
            epool = ctx.enter_context(tc.tile_pool(name="epool", bufs=e_bufs))
            work = ctx.enter_context(tc.tile_pool(name="work", bufs=work_bufs))
            outp = ctx.enter_context(tc.tile_pool(name="outp", bufs=outp_bufs))
            # PSUM (8 banks): tr 2 + mm 2x2 + acc 2x1(po|pd merged) = 8
            pp_tr = ctx.enter_context(tc.tile_pool(name="pp_tr", bufs=tr_bufs, space="PSUM"))
            pp_mm = ctx.enter_context(tc.tile_pool(name="pp_mm", bufs=mm_bufs, space="PSUM"))
            pp_acc = ctx.enter_context(tc.tile_pool(name="pp_acc", bufs=acc_bufs, space="PSUM"))

            # --- constants ---
            # f32r identity: the identity is the moving operand of a PE
            # transpose, so its dtype sets the stream rate (1.5 cyc/col vs
            # 2.0 for plain f32); transposes are exact permutations.
            ident_f = sb.tile([128, 128], F32, name="ident_f")
            make_identity(nc, ident_f)
            ident = sb.tile([128, 128], F32R, name="ident")
            nc.vector.tensor_copy(ident, ident_f)
            ones_f = sb.tile([128, 1], F32, name="ones_f")
            nc.vector.memset(ones_f, 1.0)
            ones = sb.tile([128, 2], F32R, name="ones")
            nc.vector.tensor_copy(ones[:, 0:1], ones_f)
            nc.vector.tensor_copy(ones[:, 1:2], ones_f)
            bias_m30 = sb.tile([128, 1], F32, name="bias_m30")
            nc.vector.memset(bias_m30, -TAU)

            # 0/1 causal masks for the 2 diagonal offsets of a 256-wide
            # chunk: keep q - 128*di - p >= 0 (zeroes the wedge AND the
            # fully-masked q < 128*di prefix).
            cmasks = sb.tile([128, TPC, QC], BF16, name="cmasks")
            for i in range(TPC):
                nc.vector.memset(cmasks[:, i, :], 1.0)
                nc.gpsimd.affine_select(
                    out=cmasks[:, i, :], in_=cmasks[:, i, :],
                    compare_op=mybir.AluOpType.is_ge, fill=0.0,
                    base=-128 * i, channel_multiplier=-1, pattern=[[1, QC]],
                )

            mbias = None
            if not mask_ones:
                # key-padding mask -> additive exp bias: m*10000 - 10030
                am_i = sb.tile([128, NLT], I32, name="am_i")
                nc.sync.dma_start(out=am_i, in_=am[:].rearrange("(t p) -> p t", p=128))
                am_f = sb.tile([128, NLT], F32, name="am_f")
                nc.vector.tensor_copy(am_f, am_i)
                mbias = sb.tile([128, NLT], F32, name="mbias")
                nc.vector.tensor_scalar(
                    out=mbias, in0=am_f, scalar1=-NEG_BIAS, scalar2=NEG_BIAS - TAU,
                    op0=mybir.AluOpType.mult, op1=mybir.AluOpType.add,
                )

            xT = sb.tile([128, NDT, L], F32R, name="xT")
            QT = sb.tile([128, L], F32R, name="QT")
            KT = sb.tile([128, L], F32R, name="KT")
            Vn = sb.tile([128, L], F32R, name="Vn")

            _whandles = {"q": wq, "k": wk, "v": wv}

            def w_dma(nm):
                ws = stage.tile([128, DM], F32R, name=f"ws_{nm}", tag="ws")
                nc.scalar.dma_start(out=ws, in_=_whandles[nm][:, :])
                return ws

            def w_tr(nm, ws, wTs):
                wT = sb.tile([128, NDT, 128], F32R, name=f"wT_{nm}")
                for g in range(2):
                    ps = pp_tr.tile([128, 512], F32R, name=f"ps_w{nm}{g}",
                                    tag="tr")
                    for i in range(4):
                        dt = g * 4 + i
                        nc.tensor.transpose(
                            ps[:, i * 128:(i + 1) * 128],
                            ws[:, dt * 128:(dt + 1) * 128], ident)
                    nc.vector.tensor_copy(
                        wT[:, g * 4:(g + 1) * 4, :],
                        ps.rearrange("p (a b) -> p a b", a=4))
                wTs[nm] = wT

            xs_tiles = {}

            def dma(c, split=False):
                cs = slice(c * QC, (c + 1) * QC)
                xs = stage.tile([128, TPC, DM], F32R, name="xs", tag="xs")
                if split:
                    for j in range(TPC):
                        rs = slice(c * QC + j * 128, c * QC + (j + 1) * 128)
                        nc.sync.dma_start(out=xs[:, j, :], in_=x[rs, :])
                else:
                    nc.sync.dma_start(
                        out=xs, in_=x[cs, :].rearrange("(j p) d -> p j d", p=128))
                xs_tiles[c] = xs

            def tr_piece(c, j, g):
                xs = xs_tiles[c]
                lt = TPC * c + j
                ps = pp_tr.tile([128, 512], F32R, name="ps_tr", tag="tr")
                for i in range(4):
                    dt = g * 4 + i
                    nc.tensor.transpose(
                        ps[:, i * 128:(i + 1) * 128],
                        xs[:, j, dt * 128:(dt + 1) * 128], ident)
                dst = xT[:, g * 4:(g + 1) * 4, lt * 128:(lt + 1) * 128]
                src = ps.rearrange("p (a b) -> p a b", a=4)
                nc.vector.tensor_copy(dst, src)

            def tr_pieces(c):
                return [lambda j=j, g=g: tr_piece(c, j, g)
                        for j in range(TPC) for g in range(2)]

            _vdone = set()

            def ensure_v(kc, wTs):
                if kc not in _vdone:
                    _vdone.add(kc)
                    proj_one(kc, "v", wTs)

            def proj_one(c, nm, wTs):
                cs = slice(c * QC, (c + 1) * QC)
                pm = pp_tr.tile([128, QC], F32, name=f"pm_{nm}", tag="tr")
                for dt in range(NDT):
                    nc.tensor.matmul(
                        pm, lhsT=wTs[nm][:, dt, :], rhs=xT[:, dt, cs],
                        start=(dt == 0), stop=(dt == NDT - 1),
                    )
                if nm == "v":
                    vt_c = work.tile([128, QC], F32R, name="vt_c")
                    nc.vector.tensor_copy(vt_c, pm)
                    ps = pp_tr.tile([128, QC], F32R, name="ps_vn", tag="tr")
                    for j in range(TPC):
                        nc.tensor.transpose(
                            ps[:, j * 128:(j + 1) * 128],
                            vt_c[:, j * 128:(j + 1) * 128], ident)
                    nc.vector.tensor_copy(Vn[:, cs], ps)
                else:
                    dst = (QT if nm == "q" else KT)[:, cs]
                    nc.vector.tensor_copy(dst, pm)

            # --- global (c, g) group pipeline -----------------------------
            groups = [(c, g) for c in range(NQC) for g in range(ngroups(c))]
            e_ring = {}     # (c,g) -> (e_big, nt, k0)
            acc_ring = {}   # c -> (po, pd)

            def sa(c, g):
                cs = slice(c * QC, (c + 1) * QC)
                nkt = TPC * c + TPC
                k0 = g * ab
                nt = min(ab, nkt - k0)
                pbig = pp_mm.tile([128, ab, QC], F32, name="pbig", tag="mm")
                for i in range(nt):
                    kt = k0 + i
                    nc.tensor.matmul(
                        pbig[:, i, :], lhsT=KT[:, kt * 128:(kt + 1) * 128],
                        rhs=QT[:, cs], start=True, stop=True,
                    )
                e_big = epool.tile([128, ab, QC], F32R, name="e_big")
                if inplace_exp:
                    # tanh in place on the PSUM group; the bank is then
                    # freed by the exp read instead of the tanh
                    t_big = pbig
                else:
                    t_big = tpool.tile([128, ab, QC], F32, name="t_big")
                nc.scalar.activation(
                    t_big[:, :nt, :], pbig[:, :nt, :], AF.Tanh, scale=ISQ)
                if mask_ones:
                    nc.scalar.activation(
                        e_big[:, :nt, :], t_big[:, :nt, :], AF.Exp,
                        bias=bias_m30, scale=TAU)
                else:
                    for i in range(nt):
                        kt = k0 + i
                        nc.scalar.activation(
                            e_big[:, i, :], t_big[:, i, :], AF.Exp,
                            bias=mbias[:, kt:kt + 1], scale=TAU)
                # diagonal tiles: zero the causal wedge + masked prefix
                for i in range(nt):
                    di = k0 + i - TPC * c
                    if di >= 0:
                        w = 128 * (di + 1)
                        eng = nc.gpsimd if pool_wedge else nc.vector
                        eng.tensor_mul(
                            e_big[:, i, :w], e_big[:, i, :w], cmasks[:, di, :w])
                e_ring[(c, g)] = (e_big, nt, k0)

            def bw(c, g):
                nkt = TPC * c + TPC
                e_big, nt, k0 = e_ring.pop((c, g))
                ensure_v((k0 + nt - 1) // TPC, wTs)
                if g == 0:
                    acc = pp_acc.tile([128, QC + 2 * TPC], F32, name="acc")
                    acc_ring[c] = acc
                acc = acc_ring[c]
                po = acc[:, :QC]
                pd = acc[:, QC:QC + 2 * TPC]
                for i in range(nt):
                    kt = k0 + i
                    # po and pd share one PSUM bank = one accumulation
                    # group: start on the first AV (pending-zero covers the
                    # whole bank), stop on the very last den mini-matmul.
                    nc.tensor.matmul(
                        po, lhsT=Vn[:, kt * 128:(kt + 1) * 128],
                        rhs=e_big[:, i, :],
                        start=(kt == 0), stop=False,
                    )
                    for sub in range(TPC):
                        nc.tensor.matmul(
                            pd[:, 2 * sub:2 * sub + 2],
                            lhsT=e_big[:, i, sub * 128:(sub + 1) * 128],
                            rhs=ones, start=False,
                            stop=(kt == nkt - 1 and sub == TPC - 1),
                        )
                if g == ngroups(c) - 1:
                    out_path(c)

            def out_path(c):
                cs = slice(c * QC, (c + 1) * QC)
                acc = acc_ring.pop(c)
                po = acc[:, :QC]
                pd = acc.rearrange("p (a b) -> p a b", a=(QC + 2 * TPC) // 2)
                # normalize in natural layout: den^T is already q-major
                rden = work.tile([128, TPC], F32, name="rden")
                nc.vector.reciprocal(rden, pd[:, QC // 2:QC // 2 + TPC, 0])
                on_sb = work.tile([128, QC], F32R, name="on_sb")
                ps_o = pp_tr.tile([128, QC], F32R, name="ps_o", tag="tr")
                o_sb = outp.tile([128, TPC, 128], F32, name="o_sb")
                for j in range(TPC):
                    js = slice(j * 128, (j + 1) * 128)
                    nc.vector.tensor_copy(on_sb[:, js], po[:, js])
                    nc.tensor.transpose(ps_o[:, js], on_sb[:, js], ident)
                    nc.vector.tensor_scalar_mul(
                        o_sb[:, j, :], ps_o[:, js], rden[:, j:j + 1])
                    if c == NQC - 1:
                        rs = slice(c * QC + j * 128, c * QC + (j + 1) * 128)
                        nc.gpsimd.dma_start(out=out[rs, :], in_=o_sb[:, j, :])
                if c < NQC - 1:
                    nc.gpsimd.dma_start(
                        out=out[cs, :].rearrange("(j p) d -> p j d", p=128),
                        in_=o_sb)

            # --- emission: greedy SA-ahead pipeline -----------------------
            dma(0, split=True)
            wTs = {}
            ws_q = w_dma("q")
            ws_k = w_dma("k")
            dma(1)
            ws_v = w_dma("v")
            tp0 = tr_pieces(0)
            tp0[0]()
            tp0[1]()
            w_tr("q", ws_q, wTs)
            tp0[2]()
            w_tr("k", ws_k, wTs)
            tp0[3]()
            w_tr("v", ws_v, wTs)
            dma(2)
            state = {"si": 0, "bi": 0}
            lag = lag or (e_bufs - 2)

            def sa_piece(c, g):
                while state["si"] - state["bi"] >= lag:
                    bw(*groups[state["bi"]])
                    state["bi"] += 1
                sa(c, g)
                state["si"] += 1

            def body_pieces(c):
                ps = [lambda nm=nm: proj_one(c, nm, wTs)
                      for nm in ("q", "k")]
                ps += [lambda g=g: sa_piece(c, g) for g in range(ngroups(c))]
                ps.append(lambda: ensure_v(c, wTs))
                return ps

            # interleave chunk c's proj/SA pieces with chunk c+1's x-transposes
            # so the PE->DVE->PE chains of adjacent chunks overlap.
            for c in range(NQC):
                if c + 2 < NQC:
                    dma(c + 2)
                tp = tr_pieces(c + 1) if c + 1 < NQC else []
                bp = body_pieces(c)
                n = max(len(bp), len(tp))
                for i in range(n):
                    if i < len(bp):
                        bp[i]()
                    if i < len(tp):
                        tp[i]()
            ensure_v(NQC - 2, wTs)
            ensure_v(NQC - 1, wTs)
            while state["bi"] < len(groups):
                bw(*groups[state["bi"]])
                state["bi"] += 1
    if not nc.is_finalized():
        nc.finalize()
    return nc


def _get_nc(mask_ones: bool):
    key = ("nc", mask_ones)
    if key not in _CACHE:
        _CACHE[key] = _build_nc(mask_ones, **_BUILD_OPTS)
    return _CACHE[key]


def kernel(**inputs) -> np.ndarray:
    x = np.ascontiguousarray(np.asarray(inputs["input"], dtype=np.float32))
    am = np.ascontiguousarray(np.asarray(inputs["attention_mask"], dtype=np.int32))
    wq = np.ascontiguousarray(np.asarray(inputs["W_Q"], dtype=np.float32))
    wk = np.ascontiguousarray(np.asarray(inputs["W_K"], dtype=np.float32))
    wv = np.ascontiguousarray(np.asarray(inputs["W_V"], dtype=np.float32))

    nc = _get_nc(bool((am == 1).all()))
    in_maps = [
        {"x": x[b], "attention_mask": am[b], "W_Q": wq, "W_K": wk, "W_V": wv}
        for b in range(B)
    ]
    res = run_bass_kernel_spmd(nc, in_maps, list(range(B))).results
    return np.stack([res[b]["out"] for b in range(B)]).astype(np.float32)
